# revision 1
# baseline (speedup 1.0000x reference)
"""Trainium2 Bass kernel for a 2-layer linear RNN (identity state transition).

Math: the reference computes, per layer l, h = cumsum_t(h @ W_l^T) and then
outputs = h @ W_out^T.  Cumsum along time commutes with the (time-independent)
feature matmuls, so with Wa = W1 @ W0 and Wb = W_out @ Wa:

    hidden  = cumsum_t(cumsum_t(x)) @ Wa^T
    outputs = cumsum_t(cumsum_t(x)) @ Wb^T

The double cumsum y = C^2 x has the closed form y[t] = sum_{s<=t} (t-s+1) x[s],
computed blockwise (128-step blocks) on the PE with a constant triangular
matrix T2U[s,t'] = (t'-s+1 for s<=t'), plus a 2-row carry state S = [S1; S2]:
    S1[i] = sum_{s<t0} x[s,i]          (running sum)
    S2[i] = sum_{s<t0} (t0-s) x[s,i]   (running weighted sum)
    y[t0+tau] = (x_block^T T2U)^T[tau] + S2 + (tau+1)*S1
    S1' = S1 + sum_tau x[tau];  S2' = S2 + 128*S1 + sum_tau (128-tau) x[tau]

Key layout trick: the block cumsum is computed directly TRANSPOSED —
yT_chunk = matmul(lhsT=x_chunk, rhs=T2U) gives [feature, time] chunks with no
explicit PE transposes, and the carry-add is matmul(lhsT=S_chunk, rhs=L2).
yT is exactly the operand layout the two weight matmuls need as lhsT.

Sharding: data-parallel over batch, 2 of 16 batch elements per core, weights
replicated. All matmuls run as float32r (fp32 with ~12-bit mantissa operand
reads) which streams 1 column/cycle on the PE instead of fp32's 4.

float32r matmuls lower to a fused LDWEIGHTS+MATMUL that can carry at most ONE
sync wait, so: all constants arrive in a single DMA, a warm-up dummy matmul
absorbs the constant-DMA wait, and per-subtile copy ordering keeps every f32r
matmul at <=1 un-observed dependency.
"""

import numpy as np

import concourse.bass as bass
import concourse.bacc as bacc
import concourse.mybir as mybir
from concourse.tile import TileContext
from concourse.bass_utils import run_bass_kernel_spmd

P = 128          # partitions / time-block size
H = 512          # hidden/input/output feature dim
T = 4096         # sequence length
B = 16           # batch
NCORES = 8
BPC = B // NCORES            # batch elements per core = 2
NSUB = 4                     # 128-step sub-tiles per super-tile
SUPER = P * NSUB             # 512 timesteps per DMA super-tile (1 MiB)

F32 = mybir.dt.float32
R32 = mybir.dt.float32r

# column offsets inside the packed f32r constant block
C_WA = 0
C_WB = C_WA + 4 * H
C_T2U = C_WB + 4 * H
C_L2 = C_T2U + P
C_RR = C_L2 + P
C_M2 = C_RR + 2
C_TOT = C_M2 + 2


def build_nc(bpc: int = BPC, t_len: int = T) -> bass.Bass:
    ng = t_len // SUPER      # super-tiles per batch element
    nc = bacc.Bacc(None, target_bir_lowering=False)

    x_d = nc.dram_tensor("x", [bpc * t_len, H], R32, kind="ExternalInput")
    cpack_d = nc.dram_tensor("cpack", [P, C_TOT], R32, kind="ExternalInput")
    out_d = nc.dram_tensor("outputs", [bpc * t_len, H], F32, kind="ExternalOutput")
    hid_d = nc.dram_tensor("hidden", [bpc * t_len, H], F32, kind="ExternalOutput")

    with TileContext(nc) as tc:
        with (
            tc.tile_pool(name="consts", bufs=1) as cpool,
            tc.tile_pool(name="xs", bufs=3) as xpool,
            tc.tile_pool(name="staged", bufs=3) as stpool,
            tc.tile_pool(name="ytsb", bufs=3) as ytpool_sb,
            tc.tile_pool(name="ssb", bufs=4) as spool_sb,
            tc.tile_pool(name="scratch", bufs=2) as scrpool,
            tc.tile_pool(name="psyt", bufs=2, space="PSUM") as psyt,
            tc.tile_pool(name="pss", bufs=1, space="PSUM") as pss,
            tc.tile_pool(name="pso", bufs=2, space="PSUM") as pso,
            tc.tile_pool(name="psdummy", bufs=1, space="PSUM") as psd,
        ):
            cpack = cpool.tile([P, C_TOT], R32)
            nc.sync.dma_start(out=cpack[:], in_=cpack_d[:])

            wa_sb = cpack[:, C_WA : C_WA + 4 * H]
            wb_sb = cpack[:, C_WB : C_WB + 4 * H]
            t2u_sb = cpack[:, C_T2U : C_T2U + P]
            l2_sb = cpack[0:2, C_L2 : C_L2 + P]
            rr_sb = cpack[:, C_RR : C_RR + 2]
            m2_sb = cpack[0:2, C_M2 : C_M2 + 2]

            # Warm-up: absorb the const-DMA wait so no later f32r matmul
            # needs more than one sync wait. The bank stays an open
            # accumulation group: per-super-tile absorber matmuls keep
            # accumulating into it (results never read) so they carry only
            # the x-DMA wait and no PSUM WAW self-semaphore.
            pd = psd.tile([P, P], F32, tag="pd")
            nc.tensor.matmul(
                pd[:], t2u_sb, t2u_sb, start=True, stop=False,
                skip_group_check=True,
            )

            for b in range(bpc):
                S = None  # carry state is zero at t=0: skip carry matmuls
                for g in range(ng):
                    base = b * t_len + g * SUPER
                    x_super = xpool.tile([P, NSUB, H], R32)
                    nc.sync.dma_start(
                        out=x_super[:],
                        in_=x_d[base : base + SUPER, :].rearrange(
                            "(n p) h -> p n h", p=P
                        ),
                    )
                    # absorber: observe the x-DMA on PE with a 1-element matmul
                    nc.tensor.matmul(
                        pd[0:2, 0:128], x_super[:, 0, 0:2], x_super[:, 0, 0:128],
                        start=False, stop=False, skip_group_check=True,
                    )
                    h2_super = stpool.tile([P, NSUB, H], F32, tag="h2s")
                    out_super = stpool.tile([P, NSUB, H], F32, tag="outs")
                    for n in range(NSUB):
                        x_t = x_super[:, n, :]
                        # yT chunks: [feature, time], double block-cumsum + carry
                        pyt = psyt.tile([P, H], F32)
                        for c in range(4):
                            chunk = pyt[:, c * P : (c + 1) * P]
                            nc.tensor.matmul(
                                chunk, x_t[:, c * P : (c + 1) * P], t2u_sb,
                                start=True, stop=(S is None),
                            )
                            if S is not None:
                                nc.tensor.matmul(
                                    chunk, S[:, c * P : (c + 1) * P], l2_sb,
                                    start=False, stop=True,
                                )
                        # S' = [sum x; sum (128-tau) x] + M2^T S
                        ps = pss.tile([2, H], F32)
                        nc.tensor.matmul(
                            ps[:], rr_sb, x_t, start=True, stop=(S is None)
                        )
                        if S is not None:
                            nc.tensor.matmul(
                                ps[:], m2_sb, S[:], start=False, stop=True
                            )
                        # f32r-writing copies lower to a wait-limited TR
                        # struct; tiny F32-out probes absorb the PE waits.
                        scr_s = scrpool.tile([1, 4], F32, tag="scr_s")
                        nc.scalar.copy(scr_s[0:1, 0:1], ps[0:1, 0:1])
                        S = spool_sb.tile([2, H], R32, tag="S")
                        nc.scalar.copy(S[:], ps[:])

                        scr_y = scrpool.tile([1, 4], F32, tag="scr_y")
                        nc.vector.tensor_copy(scr_y[0:1, 0:1], pyt[0:1, H - 1 : H])
                        yt_sb = ytpool_sb.tile([P, H], R32)
                        nc.vector.tensor_copy(yt_sb[:], pyt[:])

                        ph = pso.tile([P, H], F32, tag="ph")
                        po = pso.tile([P, H], F32, tag="po")
                        for c in range(4):
                            lhs = yt_sb[:, c * P : (c + 1) * P]
                            nc.tensor.matmul(
                                ph[:], lhs, wa_sb[:, c * H : (c + 1) * H],
                                start=(c == 0), stop=(c == 3),
                            )
                            nc.tensor.matmul(
                                po[:], lhs, wb_sb[:, c * H : (c + 1) * H],
                                start=(c == 0), stop=(c == 3),
                            )
                        nc.vector.tensor_copy(h2_super[:, n, :], ph[:])
                        nc.scalar.copy(out_super[:, n, :], po[:])
                    nc.sync.dma_start(
                        out=hid_d[base : base + SUPER, :].rearrange(
                            "(n p) h -> p n h", p=P
                        ),
                        in_=h2_super[:],
                    )
                    nc.sync.dma_start(
                        out=out_d[base : base + SUPER, :].rearrange(
                            "(n p) h -> p n h", p=P
                        ),
                        in_=out_super[:],
                    )
    if not nc.is_finalized():
        nc.finalize()
    return nc


def make_consts(W_ih: np.ndarray, W_out: np.ndarray) -> dict[str, np.ndarray]:
    W0 = W_ih[0].astype(np.float64)
    W1 = W_ih[1].astype(np.float64)
    Wa64 = W1 @ W0
    Wb64 = W_out.astype(np.float64) @ Wa64

    # [i, o] chunked along i into 4 partition groups -> [128, 4*512]
    def pack_w(w64):
        wT = w64.T.astype(np.float32)  # [i, o]
        return np.ascontiguousarray(
            wT.reshape(4, P, H).transpose(1, 0, 2).reshape(P, 4 * H)
        )

    tau = np.arange(P, dtype=np.float32)
    s_idx = tau[:, None]
    t_idx = tau[None, :]

    cpack = np.zeros((P, C_TOT), dtype=np.float32)
    cpack[:, C_WA : C_WA + 4 * H] = pack_w(Wa64)
    cpack[:, C_WB : C_WB + 4 * H] = pack_w(Wb64)
    cpack[:, C_T2U : C_T2U + P] = np.where(
        t_idx >= s_idx, t_idx - s_idx + 1.0, 0.0
    ).astype(np.float32)
    cpack[0, C_L2 : C_L2 + P] = tau + 1.0
    cpack[1, C_L2 : C_L2 + P] = 1.0
    cpack[:, C_RR] = 1.0
    cpack[:, C_RR + 1] = P - tau
    cpack[0, C_M2 : C_M2 + 2] = [1.0, float(P)]
    cpack[1, C_M2 : C_M2 + 2] = [0.0, 1.0]

    return {"cpack": cpack}


def kernel(x: np.ndarray, W_ih: np.ndarray, W_out: np.ndarray):
    x = np.ascontiguousarray(x, dtype=np.float32)
    consts = make_consts(np.asarray(W_ih, np.float32), np.asarray(W_out, np.float32))

    nc = build_nc()
    in_maps = []
    for core in range(NCORES):
        shard = np.ascontiguousarray(
            x[core * BPC : (core + 1) * BPC].reshape(BPC * T, H)
        )
        in_maps.append({"x": shard, **consts})

    res = run_bass_kernel_spmd(nc, in_maps, core_ids=list(range(NCORES)))
    outs = np.concatenate(
        [r["outputs"].reshape(BPC, T, H) for r in res.results], axis=0
    )
    hids = np.concatenate(
        [r["hidden"].reshape(BPC, T, H) for r in res.results], axis=0
    )
    return outs, hids



# revision 3
# speedup vs baseline: 1.6951x; 1.6951x over previous
"""Trainium2 Bass kernel for a 2-layer linear RNN (identity state transition).

Math: the reference computes, per layer l, h = cumsum_t(h @ W_l^T) and then
outputs = h @ W_out^T.  Cumsum along time commutes with the (time-independent)
feature matmuls, so with Wa = W1 @ W0 and Wb = W_out @ Wa:

    hidden  = cumsum_t(cumsum_t(x)) @ Wa^T
    outputs = cumsum_t(cumsum_t(x)) @ Wb^T

The double cumsum y = C^2 x has the closed form y[t] = sum_{s<=t} (t-s+1) x[s],
computed blockwise (128-step blocks) on the PE with a constant triangular
matrix T2U[s,t'] = (t'-s+1 for s<=t'), plus cross-block carries expressed via
raw moments U = sum_{s<t0} x[s],  V = sum_{s<t0} s*x[s]:

    y[t0+tau] = (x_block^T T2U)^T[tau] + (t0+tau+1)*U - V

U and V accumulate in an open PSUM accumulation group with ONE matmul per
block (lhsT columns [1, t0+tau]) -- no coupled state recurrence.  A [2,H]
PSUM->SBUF snapshot per block feeds the carry-add matmuls.

Key layout trick: the block cumsum is computed directly TRANSPOSED --
yT_chunk = matmul(lhsT=x_chunk, rhs=T2U) gives [feature, time] chunks with no
explicit PE transposes; the carry-add is matmul(lhsT=S_chunk, rhs=l2_g).
yT is exactly the operand layout the two weight matmuls need as lhsT.

Dtype strategy: everything on-device is float16 (inputs pre-scaled by 1/64 on
the host so the double-cumsum magnitudes stay inside fp16 range; outputs are
scaled back by 64 on the host).  fp16 matmuls run the PE at the full 2.4 GHz
warm clock with fast weight load, stream 1 column/cycle, and halve DMA and
on-chip copy traffic vs fp32.  PSUM accumulation stays fp32.  All cumsum /
carry coefficient tables (integers <= 4096) are exact or near-exact in fp16.

Sharding: data-parallel over batch, 2 of 16 batch elements per core, weights
replicated.
"""

import numpy as np

import concourse.bass as bass
import concourse.bacc as bacc
import concourse.mybir as mybir
from concourse.tile import TileContext
from concourse.bass_utils import run_bass_kernel_spmd

P = 128          # partitions / time-block size
H = 512          # hidden/input/output feature dim
T = 4096         # sequence length
B = 16           # batch
NCORES = 8
BPC = B // NCORES            # batch elements per core = 2
NSUB = 4                     # 128-step sub-tiles per super-tile
SUPER = P * NSUB             # 512 timesteps per DMA super-tile
NGB = T // P                 # 128-step blocks per batch element = 32

F32 = mybir.dt.float32
F16 = mybir.dt.float16

SCALE = 1.0 / 64.0           # host pre-scale keeping fp16 in range

# column offsets inside the packed fp16 constant block
C_WA = 0
C_WB = C_WA + 4 * H          # 2048
C_T2U = C_WB + 4 * H         # 4096
C_RR = C_T2U + P             # 4224: per-block [1, t0+tau] cols, 2 per block
C_L2 = C_RR + 2 * NGB        # 4288: per-block [t0+tau+1; -1] rows (2 parts)
C_TOT = C_L2 + NGB * P       # 8384


def build_nc(bpc: int = BPC, t_len: int = T) -> bass.Bass:
    ng = t_len // SUPER      # super-tiles per batch element
    nc = bacc.Bacc(None, target_bir_lowering=False)

    x_d = nc.dram_tensor("x", [bpc * t_len, H], F16, kind="ExternalInput")
    cpack_d = nc.dram_tensor("cpack", [P, C_TOT], F16, kind="ExternalInput")
    out_d = nc.dram_tensor("outputs", [bpc * t_len, H], F16, kind="ExternalOutput")
    hid_d = nc.dram_tensor("hidden", [bpc * t_len, H], F16, kind="ExternalOutput")

    with TileContext(nc) as tc:
        with (
            tc.tile_pool(name="consts", bufs=1) as cpool,
            tc.tile_pool(name="xs", bufs=3) as xpool,
            tc.tile_pool(name="staged", bufs=3) as stpool,
            tc.tile_pool(name="ytsb", bufs=3) as ytpool_sb,
            tc.tile_pool(name="ssb", bufs=4) as spool_sb,
            tc.tile_pool(name="psyt", bufs=2, space="PSUM") as psyt,
            tc.tile_pool(name="pss", bufs=1, space="PSUM") as pss,
            tc.tile_pool(name="pso", bufs=2, space="PSUM") as pso,
        ):
            cpack = cpool.tile([P, C_TOT], F16)
            nc.sync.dma_start(out=cpack[:], in_=cpack_d[:])

            wa_sb = cpack[:, C_WA : C_WA + 4 * H]
            wb_sb = cpack[:, C_WB : C_WB + 4 * H]
            t2u_sb = cpack[:, C_T2U : C_T2U + P]
            rr_sb = cpack[:, C_RR : C_RR + 2 * NGB]
            l2_sb = cpack[0:2, C_L2 : C_L2 + NGB * P]

            for b in range(bpc):
                psS = pss.tile([2, H], F32, tag="psS")
                S = None
                for g in range(ng):
                    base = b * t_len + g * SUPER
                    x_super = xpool.tile([P, NSUB, H], F16)
                    nc.sync.dma_start(
                        out=x_super[:],
                        in_=x_d[base : base + SUPER, :].rearrange(
                            "(n p) h -> p n h", p=P
                        ),
                    )
                    h2_super = stpool.tile([P, NSUB, H], F16, tag="h2s")
                    out_super = stpool.tile([P, NSUB, H], F16, tag="outs")
                    for n in range(NSUB):
                        gb = g * NSUB + n   # global 128-step block index
                        x_t = x_super[:, n, :]
                        if gb > 0:
                            # snapshot carry moments BEFORE this block's
                            # U/V accumulation lands in psS
                            S = spool_sb.tile([2, H], F16, tag="S")
                            nc.scalar.copy(S[:], psS[:])
                        # yT chunks: [feature, time], local double cumsum
                        # + carry (t0+tau+1)*U - V
                        pyt = psyt.tile([P, H], F32)
                        for c in range(4):
                            chunk = pyt[:, c * P : (c + 1) * P]
                            nc.tensor.matmul(
                                chunk, x_t[:, c * P : (c + 1) * P], t2u_sb,
                                start=True, stop=(S is None),
                            )
                            if S is not None:
                                nc.tensor.matmul(
                                    chunk, S[:, c * P : (c + 1) * P],
                                    l2_sb[:, gb * P : (gb + 1) * P],
                                    start=False, stop=True,
                                )
                        # accumulate U += sum x, V += sum (t0+tau)*x into psS
                        # (one matmul; start only clears on the first block,
                        # later blocks accumulate via the persistent
                        # has_written bits; each is its own closed group so
                        # the per-block snapshot read above stays legal)
                        nc.tensor.matmul(
                            psS[:], rr_sb[:, 2 * gb : 2 * gb + 2], x_t,
                            start=(gb == 0), stop=True,
                            skip_group_check=(gb > 0),
                        )
                        yt_sb = ytpool_sb.tile([P, H], F16)
                        nc.vector.tensor_copy(yt_sb[:], pyt[:])

                        ph = pso.tile([P, H], F32, tag="ph")
                        po = pso.tile([P, H], F32, tag="po")
                        for c in range(4):
                            lhs = yt_sb[:, c * P : (c + 1) * P]
                            nc.tensor.matmul(
                                ph[:], lhs, wa_sb[:, c * H : (c + 1) * H],
                                start=(c == 0), stop=(c == 3),
                            )
                            nc.tensor.matmul(
                                po[:], lhs, wb_sb[:, c * H : (c + 1) * H],
                                start=(c == 0), stop=(c == 3),
                            )
                        nc.vector.tensor_copy(h2_super[:, n, :], ph[:])
                        nc.scalar.copy(out_super[:, n, :], po[:])
                    nc.sync.dma_start(
                        out=hid_d[base : base + SUPER, :].rearrange(
                            "(n p) h -> p n h", p=P
                        ),
                        in_=h2_super[:],
                    )
                    nc.sync.dma_start(
                        out=out_d[base : base + SUPER, :].rearrange(
                            "(n p) h -> p n h", p=P
                        ),
                        in_=out_super[:],
                    )
    if not nc.is_finalized():
        nc.finalize()
    return nc


def make_consts(W_ih: np.ndarray, W_out: np.ndarray) -> dict[str, np.ndarray]:
    W0 = W_ih[0].astype(np.float64)
    W1 = W_ih[1].astype(np.float64)
    Wa64 = W1 @ W0
    Wb64 = W_out.astype(np.float64) @ Wa64

    # [i, o] chunked along i into 4 partition groups -> [128, 4*512]
    def pack_w(w64):
        wT = w64.T.astype(np.float16)  # [i, o]
        return np.ascontiguousarray(
            wT.reshape(4, P, H).transpose(1, 0, 2).reshape(P, 4 * H)
        )

    tau = np.arange(P, dtype=np.float32)
    s_idx = tau[:, None]
    t_idx = tau[None, :]

    cpack = np.zeros((P, C_TOT), dtype=np.float32)
    cpack[:, C_WA : C_WA + 4 * H] = pack_w(Wa64)
    cpack[:, C_WB : C_WB + 4 * H] = pack_w(Wb64)
    cpack[:, C_T2U : C_T2U + P] = np.where(
        t_idx >= s_idx, t_idx - s_idx + 1.0, 0.0
    )
    for gb in range(NGB):
        t0 = float(gb * P)
        cpack[:, C_RR + 2 * gb] = 1.0
        cpack[:, C_RR + 2 * gb + 1] = t0 + tau
        cpack[0, C_L2 + gb * P : C_L2 + (gb + 1) * P] = t0 + tau + 1.0
        cpack[1, C_L2 + gb * P : C_L2 + (gb + 1) * P] = -1.0

    return {"cpack": cpack.astype(np.float16)}


def make_in_maps(x: np.ndarray, W_ih: np.ndarray, W_out: np.ndarray):
    consts = make_consts(np.asarray(W_ih, np.float32), np.asarray(W_out, np.float32))
    xs = (np.asarray(x, np.float32) * SCALE).astype(np.float16)
    in_maps = []
    for core in range(NCORES):
        shard = np.ascontiguousarray(
            xs[core * BPC : (core + 1) * BPC].reshape(BPC * T, H)
        )
        in_maps.append({"x": shard, **consts})
    return in_maps


def gather_outputs(results):
    outs = np.concatenate(
        [r["outputs"].reshape(BPC, T, H).astype(np.float32) for r in results],
        axis=0,
    ) * (1.0 / SCALE)
    hids = np.concatenate(
        [r["hidden"].reshape(BPC, T, H).astype(np.float32) for r in results],
        axis=0,
    ) * (1.0 / SCALE)
    return outs, hids


def kernel(x: np.ndarray, W_ih: np.ndarray, W_out: np.ndarray):
    nc = build_nc()
    in_maps = make_in_maps(x, W_ih, W_out)
    res = run_bass_kernel_spmd(nc, in_maps, core_ids=list(range(NCORES)))
    return gather_outputs(res.results)
